# revision 1
# baseline (speedup 1.0000x reference)
"""Trainium2 Bass kernel for nn_NeuralDevice (segment_reduce).

Architecture (per reference.py):
  two "eyes": h = relu(x @ Wr + br)            [N=1M nodes, 64] -> [N, 128]
              segment-mean over idx (B=65536)  -> [B, 128]
              e = relu(mean @ Wc + bc)         -> [B, 128]
  brain:      z = [e0, e1]; out = relu(z@Wb1+bb1) @ Wb2 + bb2   -> [B, 128]

Distribution: shuffle-by-key.  Each of the 8 cores owns a contiguous range
of B/8 = 8192 segments; the host routes every node (x row + idx) of each eye
to the core owning its segment, sorted by segment, padded so that each
128-segment "window" owns a multiple of 128 stream rows (the schedule is
shared by all cores - SPMD).

On-device, per core and eye, streaming 128-row chunks:
  mm1:  h_psum[128 rows, 129] = x_aug_chunk^T(lhsT [65,128]) @ Wr_aug[65,129]
        (row 64 of x_aug is the ones row -> bias; col 128 of Wr_aug makes a
        constant 1.0 "count" column; pad rows are all-zero -> h=0, count=0)
  relu: ACT/DVE -> h_sbuf bf16 (relu keeps the 1.0 count column)
  mm2:  win_psum[128 segs, 129] += sel_chunk(lhsT [128 rows,128 segs]) @ h_sbuf
        where sel is the host-built one-hot row->segment selector (bf16).
        Segment sums and counts accumulate in PSUM f32.
When a window completes: mean = sum * 1/max(cnt,1) (per-partition scalar),
transpose to feature-major, e^T = relu(Wc^T @ mean^T + bc) -> persistent
e^T[128, 8192] bf16 per eye.  Finally the brain MLP runs feature-major over
512-segment tiles and writes out^T [128, 8192] f32; the host concatenates.
"""

import numpy as np
import ml_dtypes

from concourse import bass, mybir
import concourse.bacc as bacc
import concourse.tile as tile
from concourse.bass_utils import run_bass_kernel_spmd
from concourse.masks import make_identity

BF16 = ml_dtypes.bfloat16

# problem sizes (hardcoded per spec)
B_FULL = 65536
N_FULL = 1048576
IN_NF = 64
R_OUT = 128
C_OUT = 128
BRAIN_H = 256
BRAIN_OUT = 128

CORES = 8
P = 128
WIN = 128            # segments per accumulation window
XCHUNK = 8192        # x^T columns per DMA
SELCHUNK = 16        # 128-row chunks per selector DMA tile
HB = 3               # chunks per h-psum tile / relu batch


# ----------------------------------------------------------------- planning

def _plan(idx_eyes, segs_per_core):
    """Global window schedule + per-(eye, core) sorted node placement.

    Returns (win_sizes, total, placements):
      win_sizes[w] = padded stream rows of window w (multiple of 128),
                     shared across all cores/eyes
      placements[(e, c)] = (nodes, seg_rel) sorted by seg_rel
    """
    n_win = segs_per_core // WIN
    per_ce = {}
    runs = np.zeros((len(idx_eyes) * CORES, n_win), np.int64)
    row = 0
    for e in range(len(idx_eyes)):
        idx = idx_eyes[e]
        owner = idx // segs_per_core
        for c in range(CORES):
            nodes = np.flatnonzero(owner == c)
            seg_rel = (idx[nodes] - c * segs_per_core).astype(np.int64)
            order = np.argsort(seg_rel, kind="stable")
            nodes = nodes[order]
            seg_rel = seg_rel[order]
            per_ce[(e, c)] = (nodes, seg_rel)
            runs[row] = np.bincount(seg_rel // WIN, minlength=n_win)
            row += 1
    win_sizes = ((runs.max(axis=0) + 127) // 128) * 128
    win_sizes = np.maximum(win_sizes, 128)
    total = int(win_sizes.sum())
    return win_sizes.tolist(), total, per_ce


# ------------------------------------------------------------ program build

_NC_CACHE = {}


def _build_nc(total, win_sizes, segs):
    key = (total, tuple(win_sizes), segs)
    if key in _NC_CACHE:
        return _NC_CACHE[key]

    f32 = mybir.dt.float32
    bf16 = mybir.dt.bfloat16
    relu = mybir.ActivationFunctionType.Relu
    n_win = segs // WIN
    assert len(win_sizes) == n_win and total % 128 == 0

    nc = bacc.Bacc("TRN2", target_bir_lowering=False, debug=False)

    xT_d = [nc.dram_tensor(f"x{e}T", [IN_NF + 1, total], bf16, kind="ExternalInput")
            for e in range(2)]
    # selector, chunk-partition-major: element (p, c, s) = one-hot of row c*128+p
    sel_d = [nc.dram_tensor(f"sel{e}", [128, total // 128, WIN], bf16,
                            kind="ExternalInput") for e in range(2)]
    wr_d = [nc.dram_tensor(f"wr{e}", [IN_NF + 1, R_OUT + 1], bf16, kind="ExternalInput")
            for e in range(2)]
    wc_d = [nc.dram_tensor(f"wc{e}", [R_OUT, C_OUT], bf16, kind="ExternalInput")
            for e in range(2)]
    bc_d = [nc.dram_tensor(f"bc{e}", [C_OUT, 1], f32, kind="ExternalInput")
            for e in range(2)]
    wb1lo_d = nc.dram_tensor("wb1lo", [128, BRAIN_H], bf16, kind="ExternalInput")
    wb1hi_d = nc.dram_tensor("wb1hi", [128, BRAIN_H], bf16, kind="ExternalInput")
    bb1a_d = nc.dram_tensor("bb1a", [128, 1], f32, kind="ExternalInput")
    bb1b_d = nc.dram_tensor("bb1b", [128, 1], f32, kind="ExternalInput")
    wb2lo_d = nc.dram_tensor("wb2lo", [128, BRAIN_OUT], bf16, kind="ExternalInput")
    wb2hi_d = nc.dram_tensor("wb2hi", [128, BRAIN_OUT], bf16, kind="ExternalInput")
    bb2_d = nc.dram_tensor("bb2", [BRAIN_OUT, 1], f32, kind="ExternalInput")
    outT_d = nc.dram_tensor("outT", [128, segs], f32, kind="ExternalOutput")

    # chunk -> window map (chunk = 128 stream rows)
    win_of_chunk = []
    for w, sz in enumerate(win_sizes):
        win_of_chunk.extend([w] * (sz // 128))
    nchunks = total // 128
    assert len(win_of_chunk) == nchunks

    with tile.TileContext(nc) as tc:
        with tc.tile_pool(name="consts", bufs=1) as cp:
            ident = cp.tile([128, 128], bf16)
            make_identity(nc, ident[:])
            wr_t = [cp.tile([IN_NF + 1, R_OUT + 1], bf16, tag=f"wr{e}", name=f"wr{e}t")
                    for e in range(2)]
            wc_t = [cp.tile([R_OUT, C_OUT], bf16, tag=f"wc{e}", name=f"wc{e}t")
                    for e in range(2)]
            bc_t = [cp.tile([C_OUT, 1], f32, tag=f"bc{e}", name=f"bc{e}t")
                    for e in range(2)]
            wb1lo_t = cp.tile([128, BRAIN_H], bf16, tag="wb1lo")
            wb1hi_t = cp.tile([128, BRAIN_H], bf16, tag="wb1hi")
            bb1a_t = cp.tile([128, 1], f32, tag="bb1a")
            bb1b_t = cp.tile([128, 1], f32, tag="bb1b")
            wb2lo_t = cp.tile([128, BRAIN_OUT], bf16, tag="wb2lo")
            wb2hi_t = cp.tile([128, BRAIN_OUT], bf16, tag="wb2hi")
            bb2_t = cp.tile([BRAIN_OUT, 1], f32, tag="bb2")
            for e in range(2):
                nc.sync.dma_start(out=wr_t[e][:], in_=wr_d[e][:])
                nc.sync.dma_start(out=wc_t[e][:], in_=wc_d[e][:])
                nc.sync.dma_start(out=bc_t[e][:], in_=bc_d[e][:])
            nc.sync.dma_start(out=wb1lo_t[:], in_=wb1lo_d[:])
            nc.sync.dma_start(out=wb1hi_t[:], in_=wb1hi_d[:])
            nc.sync.dma_start(out=bb1a_t[:], in_=bb1a_d[:])
            nc.sync.dma_start(out=bb1b_t[:], in_=bb1b_d[:])
            nc.sync.dma_start(out=wb2lo_t[:], in_=wb2lo_d[:])
            nc.sync.dma_start(out=wb2hi_t[:], in_=wb2hi_d[:])
            nc.sync.dma_start(out=bb2_t[:], in_=bb2_d[:])

            eT_t = [cp.tile([128, segs], bf16, tag=f"eT{e}", name=f"eT{e}t")
                    for e in range(2)]

            # ------------------------------------------------ main phase
            with (
                tc.tile_pool(name="xch", bufs=2) as xp,
                tc.tile_pool(name="selp", bufs=3) as selp,
                tc.tile_pool(name="hs", bufs=4) as hp,
                tc.tile_pool(name="fins", bufs=3) as fs,
                tc.tile_pool(name="hps", bufs=2, space="PSUM") as hpp,
                tc.tile_pool(name="winp", bufs=3, space="PSUM") as wpp,
                tc.tile_pool(name="finp", bufs=1, space="PSUM") as fpp,
            ):
                for e in range(2):
                    xt = None
                    sel_tiles = {}
                    win_ps = None
                    chunks_left = 0
                    alt = 0
                    for c0 in range(0, nchunks, HB):
                        n = min(HB, nchunks - c0)
                        hps = hpp.tile([128, HB, 132], f32, tag="hps",
                                       name=f"hps{e}_{c0}")
                        hsb = hp.tile([128, HB, 132], bf16, tag="hs",
                                      name=f"hs{e}_{c0}")
                        for j in range(n):
                            c = c0 + j
                            if c % (XCHUNK // 128) == 0:
                                cbase = c * 128
                                csz = min(XCHUNK, total - cbase)
                                xt = xp.tile([IN_NF + 1, XCHUNK], bf16, tag="xch",
                                             name=f"xch{e}_{c}")
                                nc.sync.dma_start(out=xt[:, :csz],
                                                  in_=xT_d[e][:, cbase:cbase + csz])
                            if c % SELCHUNK == 0:
                                scnt = min(SELCHUNK, nchunks - c)
                                selt = selp.tile([128, SELCHUNK, WIN], bf16,
                                                 tag="selp", name=f"sel{e}_{c}")
                                nc.sync.dma_start(out=selt[:, :scnt, :],
                                                  in_=sel_d[e][:, c:c + scnt, :])
                                sel_tiles[c // SELCHUNK] = selt
                            xcol = (c % (XCHUNK // 128)) * 128
                            nc.tensor.matmul(
                                out=hps[:, j, 0:129],
                                lhsT=xt[0:IN_NF + 1, xcol:xcol + 128],
                                rhs=wr_t[e][:],
                                start=True, stop=True,
                            )
                        if alt % 2 == 0:
                            nc.scalar.activation(
                                out=hsb[:, :n, 0:129], in_=hps[:, :n, 0:129],
                                func=relu)
                        else:
                            nc.vector.tensor_scalar_max(
                                hsb[:, :n, 0:129], hps[:, :n, 0:129], 0.0)
                        alt += 1
                        for j in range(n):
                            c = c0 + j
                            w = win_of_chunk[c]
                            if win_ps is None:
                                win_ps = wpp.tile([128, 132], f32, tag="winp",
                                                  name=f"win{e}_{w}")
                                chunks_left = win_sizes[w] // 128
                            nc.tensor.matmul(
                                out=win_ps[:, 0:129],
                                lhsT=sel_tiles[c // SELCHUNK][:, c % SELCHUNK, :],
                                rhs=hsb[:, j, 0:129],
                                start=(chunks_left == win_sizes[w] // 128),
                                stop=(chunks_left == 1),
                            )
                            chunks_left -= 1
                            if chunks_left == 0:
                                # window complete: mean, transpose, eye linear
                                cnt = fs.tile([128, 1], f32, tag="cnt",
                                              name=f"cnt{e}_{w}")
                                rcp = fs.tile([128, 1], f32, tag="rcp",
                                              name=f"rcp{e}_{w}")
                                nc.vector.tensor_scalar_max(
                                    cnt[:], win_ps[:, 128:129], 1.0)
                                nc.vector.reciprocal(rcp[:], cnt[:])
                                meanp = fs.tile([128, 128], bf16, tag="meanp",
                                                name=f"meanp{e}_{w}")
                                nc.vector.tensor_scalar_mul(
                                    meanp[:], win_ps[:, 0:128], rcp[:, 0:1])
                                mtp = fpp.tile([128, 128], bf16, tag="mtp",
                                               name=f"mtp{e}_{w}")
                                nc.tensor.transpose(out=mtp[:], in_=meanp[:],
                                                    identity=ident[:])
                                meanT = fs.tile([128, 128], bf16, tag="meanT",
                                                name=f"meanT{e}_{w}")
                                nc.vector.tensor_copy(meanT[:], mtp[:])
                                pse = fpp.tile([128, 128], f32, tag="pse",
                                               name=f"pse{e}_{w}")
                                nc.tensor.matmul(out=pse[:], lhsT=wc_t[e][:],
                                                 rhs=meanT[:], start=True, stop=True)
                                nc.scalar.activation(
                                    out=eT_t[e][:, w * WIN:(w + 1) * WIN],
                                    in_=pse[:], func=relu, bias=bc_t[e][:, 0:1])
                                win_ps = None

            # ------------------------------------------------ brain phase
            with (
                tc.tile_pool(name="bs", bufs=3) as bs,
                tc.tile_pool(name="bph", bufs=2, space="PSUM") as bph,
                tc.tile_pool(name="bpy", bufs=2, space="PSUM") as bpy,
            ):
                for t in range(segs // 512):
                    r0 = t * 512
                    e0s = eT_t[0][:, r0:r0 + 512]
                    e1s = eT_t[1][:, r0:r0 + 512]
                    psh_a = bph.tile([128, 512], f32, tag="bph", name=f"pha{t}")
                    nc.tensor.matmul(out=psh_a[:], lhsT=wb1lo_t[:, 0:128], rhs=e0s,
                                     start=True, stop=False)
                    nc.tensor.matmul(out=psh_a[:], lhsT=wb1hi_t[:, 0:128], rhs=e1s,
                                     start=False, stop=True)
                    hTa = bs.tile([128, 512], bf16, tag="hTa", name=f"hTa{t}")
                    nc.scalar.activation(out=hTa[:], in_=psh_a[:], func=relu,
                                         bias=bb1a_t[:, 0:1])
                    psh_b = bph.tile([128, 512], f32, tag="bph", name=f"phb{t}")
                    nc.tensor.matmul(out=psh_b[:], lhsT=wb1lo_t[:, 128:256], rhs=e0s,
                                     start=True, stop=False)
                    nc.tensor.matmul(out=psh_b[:], lhsT=wb1hi_t[:, 128:256], rhs=e1s,
                                     start=False, stop=True)
                    hTb = bs.tile([128, 512], bf16, tag="hTb", name=f"hTb{t}")
                    nc.scalar.activation(out=hTb[:], in_=psh_b[:], func=relu,
                                         bias=bb1b_t[:, 0:1])
                    psy = bpy.tile([128, 512], f32, tag="bpy", name=f"py{t}")
                    nc.tensor.matmul(out=psy[:], lhsT=wb2lo_t[:], rhs=hTa[:],
                                     start=True, stop=False)
                    nc.tensor.matmul(out=psy[:], lhsT=wb2hi_t[:], rhs=hTb[:],
                                     start=False, stop=True)
                    ys = bs.tile([128, 512], f32, tag="ys", name=f"ys{t}")
                    nc.vector.tensor_scalar_add(ys[:], psy[:], bb2_t[:, 0:1])
                    nc.sync.dma_start(out=outT_d[:, r0:r0 + 512], in_=ys[:])

    nc.compile()
    _NC_CACHE[key] = nc
    return nc


# ------------------------------------------------------------------ driver

def _prepare(inputs, b_full):
    segs = b_full // CORES
    x = [np.asarray(inputs["x0"]), np.asarray(inputs["x1"])]
    idx = [np.asarray(inputs["idx0"]).astype(np.int64),
           np.asarray(inputs["idx1"]).astype(np.int64)]
    win_sizes, total, per_ce = _plan(idx, segs)
    win_base = np.cumsum([0] + win_sizes)

    shared = {}
    for e in range(2):
        wr = np.asarray(inputs[f"Wr{e}"]).astype(np.float32)
        br = np.asarray(inputs[f"br{e}"]).astype(np.float32)
        wr_aug = np.zeros((IN_NF + 1, R_OUT + 1), np.float32)
        wr_aug[:IN_NF, :R_OUT] = wr
        wr_aug[IN_NF, :R_OUT] = br
        wr_aug[IN_NF, R_OUT] = 1.0  # count column
        shared[f"wr{e}"] = wr_aug.astype(BF16)
        shared[f"wc{e}"] = np.asarray(inputs[f"Wc{e}"]).astype(BF16)
        shared[f"bc{e}"] = np.asarray(inputs[f"bc{e}"]).astype(np.float32).reshape(-1, 1)
    wb1 = np.asarray(inputs["Wb1"])
    bb1 = np.asarray(inputs["bb1"])
    wb2 = np.asarray(inputs["Wb2"])
    bb2 = np.asarray(inputs["bb2"])
    shared["wb1lo"] = wb1[0:128].astype(BF16)
    shared["wb1hi"] = wb1[128:256].astype(BF16)
    shared["bb1a"] = bb1[0:128].astype(np.float32).reshape(-1, 1)
    shared["bb1b"] = bb1[128:256].astype(np.float32).reshape(-1, 1)
    shared["wb2lo"] = wb2[0:128].astype(BF16)
    shared["wb2hi"] = wb2[128:256].astype(BF16)
    shared["bb2"] = bb2.astype(np.float32).reshape(-1, 1)

    n_win = segs // WIN
    in_maps = []
    for c in range(CORES):
        m = dict(shared)
        for e in range(2):
            nodes, seg_rel = per_ce[(e, c)]
            # stream position: window-sorted with per-window padding
            wid = seg_rel // WIN
            wstart = np.searchsorted(wid, np.arange(n_win))
            pos = np.empty(len(nodes), np.int64)
            for w in range(n_win):
                lo = wstart[w]
                hi = wstart[w + 1] if w + 1 < n_win else len(nodes)
                pos[lo:hi] = win_base[w] + np.arange(hi - lo)
            arr = np.zeros((total, IN_NF + 1), np.float32)
            arr[pos, :IN_NF] = x[e][nodes]
            arr[pos, IN_NF] = 1.0
            m[f"x{e}T"] = np.ascontiguousarray(arr.T).astype(BF16)
            sel = np.zeros((total, WIN), BF16)
            sel[pos, seg_rel % WIN] = BF16(1.0)
            # chunk-partition-major: (p, chunk, s) = sel[chunk*128 + p, s]
            m[f"sel{e}"] = np.ascontiguousarray(
                sel.reshape(total // 128, 128, WIN).transpose(1, 0, 2))
        in_maps.append(m)
    return win_sizes, total, segs, in_maps


def _axon_reset():
    try:
        import ctypes

        lib = ctypes.CDLL("/opt/axon/libaxon_pjrt.so")
        lib.axon_reset.restype = ctypes.c_int
        lib.axon_reset()
    except Exception:
        pass


def _run(inputs, trace=False, trace_kwargs=None):
    win_sizes, total, segs, in_maps = _prepare(inputs, B_FULL)
    nc = _build_nc(total, win_sizes, segs)
    try:
        res = run_bass_kernel_spmd(nc, in_maps, list(range(CORES)), trace=trace,
                                   **(trace_kwargs or {}))
    except Exception as e:
        if "UNRECOVERABLE" not in str(e) and "UNAVAILABLE" not in str(e):
            raise
        _axon_reset()
        res = run_bass_kernel_spmd(nc, in_maps, list(range(CORES)), trace=trace,
                                   **(trace_kwargs or {}))
    out = np.concatenate([res.results[c]["outT"].T for c in range(CORES)], axis=0)
    return out.astype(np.float32), res


def kernel(**inputs):
    return _run(inputs)[0]

